# revision 14
# baseline (speedup 1.0000x reference)
"""Trainium2 Bass kernel for nn_CorefMergeLayer.

Reference semantics (per example b):
    cl = m_bank[coref_posi[b], b, :]            # [K, C, H] gathered mentions
    q = cl @ Wq ; k = cl @ Uk
    scores[k,i,j] = v . tanh(q_i + k_j + b_attn)
    alpha = softmax_j(scores)
    ctx = alpha @ cl
    attn_h = [ctx; cl] @ W_out + b_out
    mrg = tanh([cl; attn_h] @ W_mrg + b_mrg)
    out = m_bank with mention rows replaced by mrg

Sharding: data-parallel over batch B=16 across 8 cores (BL=2 examples per
core); weights replicated; W_out/W_mrg/Wq/Uk streamed from HBM.
"""

import sys

for _p in ("/opt/trn_rl_repo",):
    if _p not in sys.path:
        sys.path.insert(0, _p)

import numpy as np

import concourse.bacc as bacc
import concourse.bass as bass
import concourse.mybir as mybir
import concourse.tile as tile
from concourse.bass import IndirectOffsetOnAxis
from concourse.masks import make_identity
import bass_rust as _bass_rust


def _vec_pairs(dims):
    return _bass_rust.VecI64Pair([list(d) for d in dims])


F32 = mybir.dt.float32
I32 = mybir.dt.int32
AF = mybir.ActivationFunctionType
ALU = mybir.AluOpType
AX = mybir.AxisListType

P = 128  # partitions


def build_program(S=1024, BL=2, H=1024, K=8, C=16, wdt=F32):
    """Build the SPMD per-core Bass program.

    Per-core inputs:
      mb    [S, BL, H] f32   batch slice of m_bank
      idx   [MT, BL]   i32   row indices into the (S*BL, H) view of mb
      Wq,Uk [H, H]     f32
      vattn [H]        f32
      battn,bout,bmrg [1, H] f32
      W_out,W_mrg [2H, H] f32
    Output:
      out   [S, BL, H] f32
    """
    MT = K * C                 # mentions per example (<= 128)
    M2 = BL * MT               # mention columns, both examples
    NB = H // P                # h blocks
    PAIRS_E = K * C * C        # pair columns per example
    assert MT <= P and H % P == 0 and M2 <= 512

    nc = bacc.Bacc()

    mb = nc.dram_tensor("mb", [S, BL, H], F32, kind="ExternalInput")
    idx = nc.dram_tensor("idx", [MT, BL], I32, kind="ExternalInput")
    Wq = nc.dram_tensor("Wq", [H, H], wdt, kind="ExternalInput")
    Uk = nc.dram_tensor("Uk", [H, H], wdt, kind="ExternalInput")
    vattn = nc.dram_tensor("vattn", [H], wdt, kind="ExternalInput")
    battn = nc.dram_tensor("battn", [1, H], wdt, kind="ExternalInput")
    Wout = nc.dram_tensor("Wout", [2 * H, H], wdt, kind="ExternalInput")
    bout = nc.dram_tensor("bout", [1, H], wdt, kind="ExternalInput")
    Wmrg = nc.dram_tensor("Wmrg", [2 * H, H], wdt, kind="ExternalInput")
    bmrg = nc.dram_tensor("bmrg", [1, H], wdt, kind="ExternalInput")
    out = nc.dram_tensor("out", [S, BL, H], F32, kind="ExternalOutput")

    mb2d = mb[:, :, :].rearrange("s b h -> (s b) h")
    out2d = out[:, :, :].rearrange("s b h -> (s b) h")

    with tile.TileContext(nc) as tc:
        with tc.tile_pool(name="persist", bufs=1) as pp:
            # ---- passthrough copy mb -> out (DRAM->DRAM), issued first so
            # it overlaps all compute; the final scatter depends on it (WAW).
            ncopy = 2
            rows = S // ncopy
            for i in range(ncopy):
                nc.scalar.dma_start(
                    out=out[i * rows:(i + 1) * rows, :, :],
                    in_=mb[i * rows:(i + 1) * rows, :, :],
                )

            # ---- constants into SBUF
            idx_sb = pp.tile([MT, BL], I32, tag="idx", name="idx_sb")
            nc.sync.dma_start(out=idx_sb[:, :], in_=idx[:, :])

            vT_sb = pp.tile([P, NB], wdt, tag="vT", name="vT_sb")  # vT[p, nb] = v[nb*128+p]
            nc.sync.dma_start(
                out=vT_sb[:, :], in_=vattn[:].rearrange("(nb p) -> p nb", p=P)
            )

            battn_sb = pp.tile([1, H], wdt, tag="battn", name="battn_sb")
            nc.sync.dma_start(out=battn_sb[:, :], in_=battn[:, :])
            bout_sb = pp.tile([1, H], wdt, tag="bout", name="bout_sb")
            nc.sync.dma_start(out=bout_sb[:, :], in_=bout[:, :])
            bmrg_sb = pp.tile([1, H], wdt, tag="bmrg", name="bmrg_sb")
            nc.sync.dma_start(out=bmrg_sb[:, :], in_=bmrg[:, :])

            ones_sb = pp.tile([1, max(M2, P)], wdt, tag="ones", name="ones_sb")
            nc.vector.memset(ones_sb[:, :], 1.0)

            ident = pp.tile([P, P], F32, tag="ident", name="ident")
            make_identity(nc, ident[:, :])

            # row image of the block-diagonal alphaT matrix (partition 0);
            # zeroed once early -- only the diagonal blocks are ever rewritten.
            # Shared across examples (sequential use) to save SBUF.
            diag = pp.tile([1, MT * MT], F32, tag="diag", name="diag")
            nc.vector.memset(diag[:, :], 0.0)

            # ---- gather mentions: cl[e] [MT, H] mention-major
            cl_sb = [pp.tile([MT, H], F32, tag=f"cl{e}", name=f"cl{e}") for e in range(BL)]
            for e in range(BL):
                nc.gpsimd.indirect_dma_start(
                    out=cl_sb[e][:, :],
                    out_offset=None,
                    in_=mb2d,
                    in_offset=IndirectOffsetOnAxis(ap=idx_sb[:, e:e + 1], axis=0),
                )

            # ---- transpose to clT blocks [128, M2]
            clT = [pp.tile([P, M2], wdt, tag=f"clT{b}", name=f"clT{b}") for b in range(NB)]
            with tc.tile_pool(name="tp_psum", bufs=3, space="PSUM") as tpp:
                for e in range(BL):
                    for b in range(NB):
                        tp = tpp.tile([P, MT], F32, tag="tp", name="tp")
                        nc.tensor.transpose(
                            out=tp[:, :],
                            in_=cl_sb[e][:, b * P:(b + 1) * P],
                            identity=ident[:MT, :MT],
                        )
                        nc.vector.tensor_copy(
                            out=clT[b][:, e * MT:(e + 1) * MT], in_=tp[:, :]
                        )

            # ---- projections: QT/KT blocks [128, M2];  KT += b_attn
            QT = [pp.tile([P, M2], wdt, tag=f"QT{b}", name=f"QT{b}") for b in range(NB)]
            KT = [pp.tile([P, M2], wdt, tag=f"KT{b}", name=f"KT{b}") for b in range(NB)]
            with tc.tile_pool(name="wcol", bufs=3) as wp, \
                 tc.tile_pool(name="qk_psum", bufs=2, space="PSUM") as qkp:
                for ho in range(NB):
                    # wq_col[p, hi*128 + c] = Wq[hi*128 + p, ho*128 + c]
                    wq_col = wp.tile([P, H], wdt, tag="wq", name="wq")
                    nc.sync.dma_start(
                        out=wq_col[:, :].rearrange("p (hi c) -> p hi c", hi=NB),
                        in_=Wq[:, ho * P:(ho + 1) * P].rearrange(
                            "(hi p) c -> p hi c", p=P
                        ),
                    )
                    uk_col = wp.tile([P, H], wdt, tag="uk", name="uk")
                    nc.sync.dma_start(
                        out=uk_col[:, :].rearrange("p (hi c) -> p hi c", hi=NB),
                        in_=Uk[:, ho * P:(ho + 1) * P].rearrange(
                            "(hi p) c -> p hi c", p=P
                        ),
                    )
                    qt_p = qkp.tile([P, M2], F32, tag="qt", name="qt")
                    kt_p = qkp.tile([P, M2], F32, tag="kt", name="kt")
                    for hi in range(NB):
                        nc.tensor.matmul(
                            out=qt_p[:, :],
                            lhsT=wq_col[:, hi * P:(hi + 1) * P],
                            rhs=clT[hi][:, :],
                            start=(hi == 0), stop=(hi == NB - 1),
                        )
                    # KT = b_attn (rank-1: b_chunk^T @ ones), then += Uk.T cl
                    nc.tensor.matmul(
                        out=kt_p[:, :],
                        lhsT=battn_sb[0:1, ho * P:(ho + 1) * P],
                        rhs=ones_sb[0:1, :M2],
                        start=True, stop=False,
                    )
                    for hi in range(NB):
                        nc.tensor.matmul(
                            out=kt_p[:, :],
                            lhsT=uk_col[:, hi * P:(hi + 1) * P],
                            rhs=clT[hi][:, :],
                            start=False, stop=(hi == NB - 1),
                        )
                    nc.vector.tensor_copy(out=QT[ho][:, :], in_=qt_p[:, :])
                    nc.scalar.activation(out=KT[ho][:, :], in_=kt_p[:, :], func=AF.Copy)

            # ---- pair scores: sc[e][(k,j,i)] = sum_h v[h]*tanh(q_i+k_j+b)
            NCH = (PAIRS_E + 511) // 512  # 512-col psum chunks per example
            with tc.tile_pool(name="sc_psum", bufs=1, space="PSUM") as scp, \
                 tc.tile_pool(name="epool", bufs=2) as ep:
                sc_p = [scp.tile([1, PAIRS_E], F32, tag=f"sc{e}", name=f"sc{e}") for e in range(BL)]
                for hb in range(NB):
                    for e in range(BL):
                        e_in = ep.tile([P, PAIRS_E], wdt, tag="e_in", name="e_in")
                        kin = KT[hb][:, e * MT:(e + 1) * MT] \
                            .rearrange("p (k j) -> p k j", k=K) \
                            .unsqueeze(3).to_broadcast([P, K, C, C])
                        qin = QT[hb][:, e * MT:(e + 1) * MT] \
                            .rearrange("p (k i) -> p k i", k=K) \
                            .unsqueeze(2).to_broadcast([P, K, C, C])
                        nc.vector.tensor_tensor(
                            out=e_in[:, :].rearrange("p (k j i) -> p k j i", k=K, j=C),
                            in0=kin, in1=qin, op=ALU.add,
                        )
                        e_t = ep.tile([P, PAIRS_E], wdt, tag="e_t", name="e_t", bufs=1)
                        nc.scalar.activation(out=e_t[:, :], in_=e_in[:, :], func=AF.Tanh)
                        for ch in range(NCH):
                            c0, c1 = ch * 512, min((ch + 1) * 512, PAIRS_E)
                            nc.tensor.matmul(
                                out=sc_p[e][0:1, c0:c1],
                                lhsT=vT_sb[:, hb:hb + 1],
                                rhs=e_t[:, c0:c1],
                                start=(hb == 0), stop=(hb == NB - 1),
                            )

                # ---- softmax over j; alpha laid out (k, j, i)
                abd = [pp.tile([MT, MT], F32, tag=f"abd{e}", name=f"abd{e}") for e in range(BL)]
                for e in range(BL):
                    # exp written strided into the pre-zeroed row image of the
                    # block-diagonal alphaT matrix: position of value (k,j,i)
                    # is (k*C+j)*MT + k*C + i (affine in (k,j,i)).  After the
                    # in-place normalize, one DMA reshapes the row image to
                    # the [MT, MT] tile, so the ctx matmul depends on a
                    # single DMA (HW sync-wait limit).
                    p0 = list(list(diag[0:1, :].ap)[0])
                    dg_kji = diag[0:1, :].copy()
                    dg_kji.ap = _vec_pairs([p0, [C * MT + C, K], [MT, C], [1, C]])
                    dg_kij = diag[0:1, :].copy()
                    dg_kij.ap = _vec_pairs([p0, [C * MT + C, K], [1, C], [MT, C]])
                    nc.scalar.activation(
                        out=dg_kji, in_=sc_p[e][0:1, :].rearrange(
                            "p (k j i) -> p k j i", k=K, j=C),
                        func=AF.Exp,
                    )
                    sum_sb = pp.tile([1, K * C], F32, tag=f"sum{e}", name=f"sumsb{e}")
                    nc.vector.tensor_reduce(
                        out=sum_sb[:, :], in_=dg_kij, axis=AX.X, op=ALU.add,
                    )
                    rs_sb = pp.tile([1, K * C], F32, tag=f"rs{e}", name=f"rssb{e}")
                    nc.vector.reciprocal(out=rs_sb[:, :], in_=sum_sb[:, :])
                    nc.vector.tensor_tensor(
                        out=dg_kji, in0=dg_kji,
                        in1=rs_sb[:, :].rearrange("p (k i) -> p k i", k=K)
                            .unsqueeze(2).to_broadcast([1, K, C, C]),
                        op=ALU.mult,
                    )
                    nc.sync.dma_start(out=abd[e][:, :], in_=diag[0:1, :])

            # ---- ctx: ctxT[h, (k,i)] = sum_(k,j) cl[(k,j), h] * abd[(k,j),(k,i)]
            ctxT = [pp.tile([P, M2], wdt, tag=f"ctxT{b}", name=f"ctxT{b}") for b in range(NB)]
            with tc.tile_pool(name="ctx_psum", bufs=3, space="PSUM") as cxp:
                for e in range(BL):
                    for b in range(NB):
                        cx = cxp.tile([P, MT], F32, tag="cx", name="cx")
                        nc.tensor.matmul(
                            out=cx[:, :],
                            lhsT=cl_sb[e][:, b * P:(b + 1) * P],
                            rhs=abd[e][:, :],
                            start=True, stop=True,
                        )
                        nc.vector.tensor_copy(
                            out=ctxT[b][:, e * MT:(e + 1) * MT], in_=cx[:, :]
                        )

            # ---- attn_h: attnT[d, m] = sum_f W_out[f, d] * cat1T[f, m] + b_out[d]
            # cat1T blocks: f 0..NB-1 -> ctxT, NB..2NB-1 -> clT
            attnT = [pp.tile([P, M2], wdt, tag=f"attnT{b}", name=f"attnT{b}") for b in range(NB)]
            with tc.tile_pool(name="wrow", bufs=3) as wrp, \
                 tc.tile_pool(name="at_psum", bufs=1, space="PSUM") as atp:
                at_p = [atp.tile([P, M2], F32, tag=f"at{d}", name=f"at{d}") for d in range(NB)]
                for d in range(NB):
                    nc.tensor.matmul(
                        out=at_p[d][:, :],
                        lhsT=bout_sb[0:1, d * P:(d + 1) * P],
                        rhs=ones_sb[0:1, :M2],
                        start=True, stop=False,
                    )
                for f in range(2 * NB):
                    rhs_blk = ctxT[f] if f < NB else clT[f - NB]
                    w_row = wrp.tile([P, H], wdt, tag="wout", name="wout")
                    nc.sync.dma_start(
                        out=w_row[:, :], in_=Wout[f * P:(f + 1) * P, :]
                    )
                    for d in range(NB):
                        nc.tensor.matmul(
                            out=at_p[d][:, :],
                            lhsT=w_row[:, d * P:(d + 1) * P],
                            rhs=rhs_blk[:, :],
                            start=False, stop=(f == 2 * NB - 1),
                        )
                for d in range(NB):
                    nc.vector.tensor_copy(out=attnT[d][:, :], in_=at_p[d][:, :])

            # ---- mrg: mrg[m, d] = tanh(sum_f cat2T[f, m] * W_mrg[f, d] + b_mrg[d])
            # cat2T blocks: f 0..NB-1 -> clT, NB..2NB-1 -> attnT
            mrg_sb = [pp.tile([MT, H], F32, tag=f"mrg{e}", name=f"mrg{e}") for e in range(BL)]
            ND2 = H // 512 if H >= 512 else 1
            DW = min(H, 512)
            with tc.tile_pool(name="wrow2", bufs=3) as wr2, \
                 tc.tile_pool(name="mg_psum", bufs=1, space="PSUM") as mgp:
                mg_p = [mgp.tile([MT, H], F32, tag=f"mg{e}", name=f"mg{e}") for e in range(BL)]
                for e in range(BL):
                    for d2 in range(ND2):
                        nc.tensor.matmul(
                            out=mg_p[e][:, d2 * DW:(d2 + 1) * DW],
                            lhsT=ones_sb[0:1, :MT],
                            rhs=bmrg_sb[0:1, d2 * DW:(d2 + 1) * DW],
                            start=True, stop=False,
                        )
                for f in range(2 * NB):
                    lhs_blk = clT[f] if f < NB else attnT[f - NB]
                    w_row = wr2.tile([P, H], wdt, tag="wmrg", name="wmrg")
                    nc.sync.dma_start(
                        out=w_row[:, :], in_=Wmrg[f * P:(f + 1) * P, :]
                    )
                    for e in range(BL):
                        for d2 in range(ND2):
                            nc.tensor.matmul(
                                out=mg_p[e][:, d2 * DW:(d2 + 1) * DW],
                                lhsT=lhs_blk[:, e * MT:(e + 1) * MT],
                                rhs=w_row[:, d2 * DW:(d2 + 1) * DW],
                                start=False, stop=(f == 2 * NB - 1),
                            )
                for e in range(BL):
                    nc.scalar.activation(
                        out=mrg_sb[e][:, :], in_=mg_p[e][:, :], func=AF.Tanh
                    )

            # ---- scatter merged rows into out (after passthrough copy: WAW)
            for e in range(BL):
                nc.gpsimd.indirect_dma_start(
                    out=out2d,
                    out_offset=IndirectOffsetOnAxis(ap=idx_sb[:, e:e + 1], axis=0),
                    in_=mrg_sb[e][:, :],
                    in_offset=None,
                )

    return nc


# ---------------------------------------------------------------------------

S, B, H, K, C = 1024, 16, 1024, 8, 16
N_CORES = 8
BL = B // N_CORES
WEIGHT_DTYPE = F32  # set to mybir.dt.bfloat16 to halve weight HBM traffic

_prog_cache = {}


def _np_wdt():
    return mybir.dt.np(WEIGHT_DTYPE)


def _get_program():
    key = (S, BL, H, K, C, WEIGHT_DTYPE)
    if key not in _prog_cache:
        nc = build_program(S, BL, H, K, C, wdt=WEIGHT_DTYPE)
        nc.finalize()  # Bacc.finalize: wait-splitting, reg alloc, codegen
        _prog_cache[key] = nc
    return _prog_cache[key]


def make_in_maps(m_bank, coref_posi, Wq, Uk, b_attn, v_attn, W_out, b_out,
                 W_mrg, b_mrg):
    MT = K * C
    m_bank = np.ascontiguousarray(m_bank, dtype=np.float32)
    in_maps = []
    for c in range(N_CORES):
        mb_c = np.ascontiguousarray(m_bank[:, c * BL:(c + 1) * BL, :])
        # idx[m, e]: row of mention m of local example e in the (S*BL, H) view
        idx_c = np.empty((MT, BL), dtype=np.int32)
        for e in range(BL):
            pos = np.asarray(coref_posi[c * BL + e], dtype=np.int64).reshape(MT)
            idx_c[:, e] = (pos * BL + e).astype(np.int32)
        in_maps.append({
            "mb": mb_c,
            "idx": idx_c,
            "Wq": np.ascontiguousarray(Wq, dtype=_np_wdt()),
            "Uk": np.ascontiguousarray(Uk, dtype=_np_wdt()),
            "vattn": np.ascontiguousarray(v_attn, dtype=_np_wdt()).reshape(H),
            "battn": np.ascontiguousarray(b_attn, dtype=_np_wdt()).reshape(1, H),
            "Wout": np.ascontiguousarray(W_out, dtype=_np_wdt()),
            "bout": np.ascontiguousarray(b_out, dtype=_np_wdt()).reshape(1, H),
            "Wmrg": np.ascontiguousarray(W_mrg, dtype=_np_wdt()),
            "bmrg": np.ascontiguousarray(b_mrg, dtype=_np_wdt()).reshape(1, H),
        })
    return in_maps


def run(in_maps, trace=False, tmpdir=None):
    from concourse.bass_utils import run_bass_kernel_spmd
    nc = _get_program()
    return run_bass_kernel_spmd(
        nc, in_maps, list(range(N_CORES)), trace=trace, tmpdir=tmpdir
    )


def kernel(**inputs):
    in_maps = make_in_maps(**inputs)
    res = run(in_maps)
    outs = [res.results[c]["out"] for c in range(N_CORES)]
    return np.concatenate(outs, axis=1).astype(np.float32)


if __name__ == "__main__":
    nc = build_program()
    print("program built ok; instructions:",
          sum(len(bb.instructions) for f in nc.m.functions for bb in f.basicblocks)
          if hasattr(nc.m.functions[0], "basicblocks") else "n/a")


# revision 15
# speedup vs baseline: 1.7080x; 1.7080x over previous
"""Trainium2 Bass kernel for nn_CorefMergeLayer.

Reference semantics (per example b):
    cl = m_bank[coref_posi[b], b, :]            # [K, C, H] gathered mentions
    q = cl @ Wq ; k = cl @ Uk
    scores[k,i,j] = v . tanh(q_i + k_j + b_attn)
    alpha = softmax_j(scores)
    ctx = alpha @ cl
    attn_h = [ctx; cl] @ W_out + b_out
    mrg = tanh([cl; attn_h] @ W_mrg + b_mrg)
    out = m_bank with mention rows replaced by mrg

Sharding: data-parallel over batch B=16 across 8 cores (BL=2 examples per
core); weights replicated; W_out/W_mrg/Wq/Uk streamed from HBM.
"""

import sys

for _p in ("/opt/trn_rl_repo",):
    if _p not in sys.path:
        sys.path.insert(0, _p)

import numpy as np

import concourse.bacc as bacc
import concourse.bass as bass
import concourse.mybir as mybir
import concourse.tile as tile
from concourse.bass import IndirectOffsetOnAxis
from concourse.masks import make_identity
import bass_rust as _bass_rust


def _vec_pairs(dims):
    return _bass_rust.VecI64Pair([list(d) for d in dims])


F32 = mybir.dt.float32
I32 = mybir.dt.int32
AF = mybir.ActivationFunctionType
ALU = mybir.AluOpType
AX = mybir.AxisListType

P = 128  # partitions


def build_program(S=1024, BL=2, H=1024, K=8, C=16, wdt=F32):
    """Build the SPMD per-core Bass program.

    Per-core inputs:
      mb    [S, BL, H] f32   batch slice of m_bank
      idx   [MT, BL]   i32   row indices into the (S*BL, H) view of mb
      Wq,Uk [H, H]     f32
      vattn [H]        f32
      battn,bout,bmrg [1, H] f32
      W_out,W_mrg [2H, H] f32
    Output:
      out   [S, BL, H] f32
    """
    MT = K * C                 # mentions per example (<= 128)
    M2 = BL * MT               # mention columns, both examples
    NB = H // P                # h blocks
    PAIRS_E = K * C * C        # pair columns per example
    assert MT <= P and H % P == 0 and M2 <= 512

    nc = bacc.Bacc()

    mb = nc.dram_tensor("mb", [S, BL, H], F32, kind="ExternalInput")
    idx = nc.dram_tensor("idx", [MT, BL], I32, kind="ExternalInput")
    Wq = nc.dram_tensor("Wq", [H, H], wdt, kind="ExternalInput")
    Uk = nc.dram_tensor("Uk", [H, H], wdt, kind="ExternalInput")
    vattn = nc.dram_tensor("vattn", [H], wdt, kind="ExternalInput")
    battn = nc.dram_tensor("battn", [1, H], wdt, kind="ExternalInput")
    Wout = nc.dram_tensor("Wout", [2 * H, H], wdt, kind="ExternalInput")
    bout = nc.dram_tensor("bout", [1, H], wdt, kind="ExternalInput")
    Wmrg = nc.dram_tensor("Wmrg", [2 * H, H], wdt, kind="ExternalInput")
    bmrg = nc.dram_tensor("bmrg", [1, H], wdt, kind="ExternalInput")
    out = nc.dram_tensor("out", [S, BL, H], F32, kind="ExternalOutput")

    mb2d = mb[:, :, :].rearrange("s b h -> (s b) h")
    out2d = out[:, :, :].rearrange("s b h -> (s b) h")

    with tile.TileContext(nc) as tc:
        with tc.tile_pool(name="persist", bufs=1) as pp:
            # ---- passthrough copy mb -> out (DRAM->DRAM), issued first so
            # it overlaps all compute; the final scatter depends on it (WAW).
            ncopy = 2
            rows = S // ncopy
            for i in range(ncopy):
                nc.scalar.dma_start(
                    out=out[i * rows:(i + 1) * rows, :, :],
                    in_=mb[i * rows:(i + 1) * rows, :, :],
                )

            # ---- constants into SBUF
            idx_sb = pp.tile([MT, BL], I32, tag="idx", name="idx_sb")
            nc.sync.dma_start(out=idx_sb[:, :], in_=idx[:, :])

            vT_sb = pp.tile([P, NB], wdt, tag="vT", name="vT_sb")  # vT[p, nb] = v[nb*128+p]
            nc.sync.dma_start(
                out=vT_sb[:, :], in_=vattn[:].rearrange("(nb p) -> p nb", p=P)
            )

            battn_sb = pp.tile([1, H], wdt, tag="battn", name="battn_sb")
            nc.sync.dma_start(out=battn_sb[:, :], in_=battn[:, :])
            bout_sb = pp.tile([1, H], wdt, tag="bout", name="bout_sb")
            nc.sync.dma_start(out=bout_sb[:, :], in_=bout[:, :])
            bmrg_sb = pp.tile([1, H], wdt, tag="bmrg", name="bmrg_sb")
            nc.sync.dma_start(out=bmrg_sb[:, :], in_=bmrg[:, :])

            ones_sb = pp.tile([1, max(M2, P)], wdt, tag="ones", name="ones_sb")
            nc.vector.memset(ones_sb[:, :], 1.0)

            ident = pp.tile([P, P], F32, tag="ident", name="ident")
            make_identity(nc, ident[:, :])

            # row image of the block-diagonal alphaT matrix (partition 0);
            # zeroed once early -- only the diagonal blocks are ever rewritten.
            # Shared across examples (sequential use) to save SBUF.
            diag = pp.tile([1, MT * MT], F32, tag="diag", name="diag")
            nc.vector.memset(diag[:, :], 0.0)

            # ---- gather mentions: cl[e] [MT, H] mention-major
            cl_sb = [pp.tile([MT, H], F32, tag=f"cl{e}", name=f"cl{e}") for e in range(BL)]
            for e in range(BL):
                nc.gpsimd.indirect_dma_start(
                    out=cl_sb[e][:, :],
                    out_offset=None,
                    in_=mb2d,
                    in_offset=IndirectOffsetOnAxis(ap=idx_sb[:, e:e + 1], axis=0),
                )

            # ---- transpose to clT blocks [128, M2]
            clT = [pp.tile([P, M2], wdt, tag=f"clT{b}", name=f"clT{b}") for b in range(NB)]
            with tc.tile_pool(name="tp_psum", bufs=3, space="PSUM") as tpp:
                for e in range(BL):
                    for b in range(NB):
                        tp = tpp.tile([P, MT], F32, tag="tp", name="tp")
                        nc.tensor.transpose(
                            out=tp[:, :],
                            in_=cl_sb[e][:, b * P:(b + 1) * P],
                            identity=ident[:MT, :MT],
                        )
                        nc.vector.tensor_copy(
                            out=clT[b][:, e * MT:(e + 1) * MT], in_=tp[:, :]
                        )

            # ---- projections: QT/KT blocks [128, M2];  KT += b_attn
            QT = [pp.tile([P, M2], wdt, tag=f"QT{b}", name=f"QT{b}") for b in range(NB)]
            KT = [pp.tile([P, M2], wdt, tag=f"KT{b}", name=f"KT{b}") for b in range(NB)]
            with tc.tile_pool(name="wcol", bufs=3) as wp, \
                 tc.tile_pool(name="qk_psum", bufs=2, space="PSUM") as qkp:
                for ho in range(NB):
                    # wq_col[p, hi*128 + c] = Wq[hi*128 + p, ho*128 + c]
                    wq_col = wp.tile([P, H], wdt, tag="wq", name="wq")
                    nc.sync.dma_start(
                        out=wq_col[:, :].rearrange("p (hi c) -> p hi c", hi=NB),
                        in_=Wq[:, ho * P:(ho + 1) * P].rearrange(
                            "(hi p) c -> p hi c", p=P
                        ),
                    )
                    uk_col = wp.tile([P, H], wdt, tag="uk", name="uk")
                    nc.sync.dma_start(
                        out=uk_col[:, :].rearrange("p (hi c) -> p hi c", hi=NB),
                        in_=Uk[:, ho * P:(ho + 1) * P].rearrange(
                            "(hi p) c -> p hi c", p=P
                        ),
                    )
                    qt_p = qkp.tile([P, M2], F32, tag="qt", name="qt")
                    kt_p = qkp.tile([P, M2], F32, tag="kt", name="kt")
                    for hi in range(NB):
                        nc.tensor.matmul(
                            out=qt_p[:, :],
                            lhsT=wq_col[:, hi * P:(hi + 1) * P],
                            rhs=clT[hi][:, :],
                            start=(hi == 0), stop=(hi == NB - 1),
                        )
                    # KT = b_attn (rank-1: b_chunk^T @ ones), then += Uk.T cl
                    nc.tensor.matmul(
                        out=kt_p[:, :],
                        lhsT=battn_sb[0:1, ho * P:(ho + 1) * P],
                        rhs=ones_sb[0:1, :M2],
                        start=True, stop=False,
                    )
                    for hi in range(NB):
                        nc.tensor.matmul(
                            out=kt_p[:, :],
                            lhsT=uk_col[:, hi * P:(hi + 1) * P],
                            rhs=clT[hi][:, :],
                            start=False, stop=(hi == NB - 1),
                        )
                    nc.vector.tensor_copy(out=QT[ho][:, :], in_=qt_p[:, :])
                    nc.scalar.activation(out=KT[ho][:, :], in_=kt_p[:, :], func=AF.Copy)

            # ---- pair scores: sc[e][(k,j,i)] = sum_h v[h]*tanh(q_i+k_j+b)
            NCH = (PAIRS_E + 511) // 512  # 512-col psum chunks per example
            with tc.tile_pool(name="sc_psum", bufs=1, space="PSUM") as scp, \
                 tc.tile_pool(name="epool", bufs=2) as ep:
                sc_p = [scp.tile([1, PAIRS_E], F32, tag=f"sc{e}", name=f"sc{e}") for e in range(BL)]
                for hb in range(NB):
                    for e in range(BL):
                        e_in = ep.tile([P, PAIRS_E], wdt, tag="e_in", name="e_in")
                        kin = KT[hb][:, e * MT:(e + 1) * MT] \
                            .rearrange("p (k j) -> p k j", k=K) \
                            .unsqueeze(3).to_broadcast([P, K, C, C])
                        qin = QT[hb][:, e * MT:(e + 1) * MT] \
                            .rearrange("p (k i) -> p k i", k=K) \
                            .unsqueeze(2).to_broadcast([P, K, C, C])
                        nc.vector.tensor_tensor(
                            out=e_in[:, :].rearrange("p (k j i) -> p k j i", k=K, j=C),
                            in0=kin, in1=qin, op=ALU.add,
                        )
                        e_t = ep.tile([P, PAIRS_E], wdt, tag="e_t", name="e_t", bufs=1)
                        nc.scalar.activation(out=e_t[:, :], in_=e_in[:, :], func=AF.Tanh)
                        for ch in range(NCH):
                            c0, c1 = ch * 512, min((ch + 1) * 512, PAIRS_E)
                            nc.tensor.matmul(
                                out=sc_p[e][0:1, c0:c1],
                                lhsT=vT_sb[:, hb:hb + 1],
                                rhs=e_t[:, c0:c1],
                                start=(hb == 0), stop=(hb == NB - 1),
                            )

                # ---- softmax over j; alpha laid out (k, j, i)
                abd = [pp.tile([MT, MT], F32, tag=f"abd{e}", name=f"abd{e}") for e in range(BL)]
                for e in range(BL):
                    # exp written strided into the pre-zeroed row image of the
                    # block-diagonal alphaT matrix: position of value (k,j,i)
                    # is (k*C+j)*MT + k*C + i (affine in (k,j,i)).  After the
                    # in-place normalize, one DMA reshapes the row image to
                    # the [MT, MT] tile, so the ctx matmul depends on a
                    # single DMA (HW sync-wait limit).
                    p0 = list(list(diag[0:1, :].ap)[0])
                    dg_kji = diag[0:1, :].copy()
                    dg_kji.ap = _vec_pairs([p0, [C * MT + C, K], [MT, C], [1, C]])
                    dg_kij = diag[0:1, :].copy()
                    dg_kij.ap = _vec_pairs([p0, [C * MT + C, K], [1, C], [MT, C]])
                    nc.scalar.activation(
                        out=dg_kji, in_=sc_p[e][0:1, :].rearrange(
                            "p (k j i) -> p k j i", k=K, j=C),
                        func=AF.Exp,
                    )
                    sum_sb = pp.tile([1, K * C], F32, tag=f"sum{e}", name=f"sumsb{e}")
                    nc.vector.tensor_reduce(
                        out=sum_sb[:, :], in_=dg_kij, axis=AX.X, op=ALU.add,
                    )
                    rs_sb = pp.tile([1, K * C], F32, tag=f"rs{e}", name=f"rssb{e}")
                    nc.vector.reciprocal(out=rs_sb[:, :], in_=sum_sb[:, :])
                    nc.vector.tensor_tensor(
                        out=dg_kji, in0=dg_kji,
                        in1=rs_sb[:, :].rearrange("p (k i) -> p k i", k=K)
                            .unsqueeze(2).to_broadcast([1, K, C, C]),
                        op=ALU.mult,
                    )
                    nc.sync.dma_start(out=abd[e][:, :], in_=diag[0:1, :])

            # ---- ctx: ctxT[h, (k,i)] = sum_(k,j) cl[(k,j), h] * abd[(k,j),(k,i)]
            ctxT = [pp.tile([P, M2], wdt, tag=f"ctxT{b}", name=f"ctxT{b}") for b in range(NB)]
            with tc.tile_pool(name="ctx_psum", bufs=3, space="PSUM") as cxp:
                for e in range(BL):
                    for b in range(NB):
                        cx = cxp.tile([P, MT], F32, tag="cx", name="cx")
                        nc.tensor.matmul(
                            out=cx[:, :],
                            lhsT=cl_sb[e][:, b * P:(b + 1) * P],
                            rhs=abd[e][:, :],
                            start=True, stop=True,
                        )
                        nc.vector.tensor_copy(
                            out=ctxT[b][:, e * MT:(e + 1) * MT], in_=cx[:, :]
                        )

            # ---- attn_h: attnT[d, m] = sum_f W_out[f, d] * cat1T[f, m] + b_out[d]
            # cat1T blocks: f 0..NB-1 -> ctxT, NB..2NB-1 -> clT
            attnT = [pp.tile([P, M2], wdt, tag=f"attnT{b}", name=f"attnT{b}") for b in range(NB)]
            with tc.tile_pool(name="wrow", bufs=3) as wrp, \
                 tc.tile_pool(name="at_psum", bufs=1, space="PSUM") as atp:
                at_p = [atp.tile([P, M2], F32, tag=f"at{d}", name=f"at{d}") for d in range(NB)]
                for d in range(NB):
                    nc.tensor.matmul(
                        out=at_p[d][:, :],
                        lhsT=bout_sb[0:1, d * P:(d + 1) * P],
                        rhs=ones_sb[0:1, :M2],
                        start=True, stop=False,
                    )
                for f in range(2 * NB):
                    rhs_blk = ctxT[f] if f < NB else clT[f - NB]
                    w_row = wrp.tile([P, H], wdt, tag="wout", name="wout")
                    nc.sync.dma_start(
                        out=w_row[:, :], in_=Wout[f * P:(f + 1) * P, :]
                    )
                    for d in range(NB):
                        nc.tensor.matmul(
                            out=at_p[d][:, :],
                            lhsT=w_row[:, d * P:(d + 1) * P],
                            rhs=rhs_blk[:, :],
                            start=False, stop=(f == 2 * NB - 1),
                        )
                for d in range(NB):
                    nc.vector.tensor_copy(out=attnT[d][:, :], in_=at_p[d][:, :])

            # ---- mrg: mrg[m, d] = tanh(sum_f cat2T[f, m] * W_mrg[f, d] + b_mrg[d])
            # cat2T blocks: f 0..NB-1 -> clT, NB..2NB-1 -> attnT
            mrg_sb = [pp.tile([MT, H], F32, tag=f"mrg{e}", name=f"mrg{e}") for e in range(BL)]
            ND2 = H // 512 if H >= 512 else 1
            DW = min(H, 512)
            with tc.tile_pool(name="wrow2", bufs=3) as wr2, \
                 tc.tile_pool(name="mg_psum", bufs=1, space="PSUM") as mgp:
                mg_p = [mgp.tile([MT, H], F32, tag=f"mg{e}", name=f"mg{e}") for e in range(BL)]
                for e in range(BL):
                    for d2 in range(ND2):
                        nc.tensor.matmul(
                            out=mg_p[e][:, d2 * DW:(d2 + 1) * DW],
                            lhsT=ones_sb[0:1, :MT],
                            rhs=bmrg_sb[0:1, d2 * DW:(d2 + 1) * DW],
                            start=True, stop=False,
                        )
                for f in range(2 * NB):
                    lhs_blk = clT[f] if f < NB else attnT[f - NB]
                    w_row = wr2.tile([P, H], wdt, tag="wmrg", name="wmrg")
                    nc.sync.dma_start(
                        out=w_row[:, :], in_=Wmrg[f * P:(f + 1) * P, :]
                    )
                    for e in range(BL):
                        for d2 in range(ND2):
                            nc.tensor.matmul(
                                out=mg_p[e][:, d2 * DW:(d2 + 1) * DW],
                                lhsT=lhs_blk[:, e * MT:(e + 1) * MT],
                                rhs=w_row[:, d2 * DW:(d2 + 1) * DW],
                                start=False, stop=(f == 2 * NB - 1),
                            )
                for e in range(BL):
                    nc.scalar.activation(
                        out=mrg_sb[e][:, :], in_=mg_p[e][:, :], func=AF.Tanh
                    )

            # ---- scatter merged rows into out (after passthrough copy: WAW)
            for e in range(BL):
                nc.gpsimd.indirect_dma_start(
                    out=out2d,
                    out_offset=IndirectOffsetOnAxis(ap=idx_sb[:, e:e + 1], axis=0),
                    in_=mrg_sb[e][:, :],
                    in_offset=None,
                )

    return nc


# ---------------------------------------------------------------------------

S, B, H, K, C = 1024, 16, 1024, 8, 16
N_CORES = 8
BL = B // N_CORES
WEIGHT_DTYPE = mybir.dt.bfloat16  # F32 for exact; bf16 halves weight HBM + 2x PE

_prog_cache = {}


def _np_wdt():
    return mybir.dt.np(WEIGHT_DTYPE)


def _get_program():
    key = (S, BL, H, K, C, WEIGHT_DTYPE)
    if key not in _prog_cache:
        nc = build_program(S, BL, H, K, C, wdt=WEIGHT_DTYPE)
        nc.finalize()  # Bacc.finalize: wait-splitting, reg alloc, codegen
        _prog_cache[key] = nc
    return _prog_cache[key]


def make_in_maps(m_bank, coref_posi, Wq, Uk, b_attn, v_attn, W_out, b_out,
                 W_mrg, b_mrg):
    MT = K * C
    m_bank = np.ascontiguousarray(m_bank, dtype=np.float32)
    in_maps = []
    for c in range(N_CORES):
        mb_c = np.ascontiguousarray(m_bank[:, c * BL:(c + 1) * BL, :])
        # idx[m, e]: row of mention m of local example e in the (S*BL, H) view
        idx_c = np.empty((MT, BL), dtype=np.int32)
        for e in range(BL):
            pos = np.asarray(coref_posi[c * BL + e], dtype=np.int64).reshape(MT)
            idx_c[:, e] = (pos * BL + e).astype(np.int32)
        in_maps.append({
            "mb": mb_c,
            "idx": idx_c,
            "Wq": np.ascontiguousarray(Wq, dtype=_np_wdt()),
            "Uk": np.ascontiguousarray(Uk, dtype=_np_wdt()),
            "vattn": np.ascontiguousarray(v_attn, dtype=_np_wdt()).reshape(H),
            "battn": np.ascontiguousarray(b_attn, dtype=_np_wdt()).reshape(1, H),
            "Wout": np.ascontiguousarray(W_out, dtype=_np_wdt()),
            "bout": np.ascontiguousarray(b_out, dtype=_np_wdt()).reshape(1, H),
            "Wmrg": np.ascontiguousarray(W_mrg, dtype=_np_wdt()),
            "bmrg": np.ascontiguousarray(b_mrg, dtype=_np_wdt()).reshape(1, H),
        })
    return in_maps


def run(in_maps, trace=False, tmpdir=None):
    from concourse.bass_utils import run_bass_kernel_spmd
    nc = _get_program()
    return run_bass_kernel_spmd(
        nc, in_maps, list(range(N_CORES)), trace=trace, tmpdir=tmpdir
    )


def kernel(**inputs):
    in_maps = make_in_maps(**inputs)
    res = run(in_maps)
    outs = [res.results[c]["out"] for c in range(N_CORES)]
    return np.concatenate(outs, axis=1).astype(np.float32)


if __name__ == "__main__":
    nc = build_program()
    print("program built ok; instructions:",
          sum(len(bb.instructions) for f in nc.m.functions for bb in f.basicblocks)
          if hasattr(nc.m.functions[0], "basicblocks") else "n/a")
